# revision 41
# baseline (speedup 1.0000x reference)
"""Trainium2 Bass kernel: PhraseEncodingMixin pairwise span means.

out[b, i, j, h] = (c[b, j, h] - cp[b, i, h]) / (|i - j| + 1)
  c  = cumsum(seq_hiddens, axis=1)
  cp = c shifted right one step (zero padded)

Strategy (8 NeuronCores, i-axis sharded, 32 span-starts per core):
  - r[i, j] = 1 / (|i-j| + 1) precomputed on host (tiny tables).
  - Per (i, j-block): PE computes the rank-1 term r[i, jblk] (x) cp[b, i, :]
    into PSUM with K=32 fp32 selector matmuls (row il of the stationary is
    the r-row, other rows zero; base partition must be 0), batched over all
    4 batch entries (b-major free layout keeps the per-partition scalar
    fixed per op).
  - One fused DVE scalar_tensor_tensor per (i, j-block):
        out = (c * r_col) - psum        # single full-rate pass
  - Output DRAM layout mirrors the SBUF staging layout, so the store DMAs
    are fully contiguous 4 MiB transfers (4 span rows per DMA); the host
    transposes back to [B, L, L, H] when gathering the 8 shards.
"""

import sys

sys.path.insert(0, "/opt/trn_rl_repo")

import numpy as np

B, L, H = 4, 256, 256
NCORES = 8
NI = L // NCORES  # span-start rows per core (32)
P = 128           # SBUF partitions
JB = L // P       # j blocks (2)
BH = B * H        # 1024

IG = 4            # span rows per output DMA group
PADW = 4          # pad columns per staging tile (guard-copy target)
CW = IG * JB * BH + PADW  # staging tile width
_cache = {}


def _build_program():
    import concourse.bass as bass
    import concourse.mybir as mybir
    from concourse import tile
    from contextlib import ExitStack

    f32 = mybir.dt.float32
    Alu = mybir.AluOpType

    nc = bass.Bass("TRN2", target_bir_lowering=False, debug=False)

    # cmat = [c | rcol] packed: cmat[p, jb*BH + b*H + h] = c[b, jb*P+p, h],
    # cmat[p, JB*BH + il*JB+jb] = r[i0+il, jb*P+p]. One DMA -> one semaphore
    # source for the DVE ops (walrus limits sync waits per instruction).
    cmat = nc.dram_tensor(
        "cmat", [P, JB * BH + NI * JB], f32, kind="ExternalInput"
    ).ap()
    # wsrc = [cpf | rsel] packed so the PE operands arrive via ONE DMA (the
    # fp32 matmul's LW struct only tolerates a single sync wait).
    wsrc = nc.dram_tensor(
        "wsrc", [NI, BH + NI * JB * P], f32, kind="ExternalInput"
    ).ap()
    # out[ig, p, il'*2048 + b*512 + jb*256 + h] — verbatim staging tiles
    # (each group's 4 pad columns included; host drops them)
    out = nc.dram_tensor(
        "out", [NI // IG, P, CW], f32, kind="ExternalOutput"
    ).ap()

    with tile.TileContext(nc) as tc:
        with ExitStack() as ctx:
            const = ctx.enter_context(tc.tile_pool(name="const", bufs=1))
            psum_pool = ctx.enter_context(
                tc.tile_pool(name="psum", bufs=4, space="PSUM")
            )
            stag_pool = ctx.enter_context(tc.tile_pool(name="stag", bufs=3))

            # cumsum tiles, j on partitions, columns [jb][b][h], then rcol
            cmat_sb = const.tile([P, JB * BH + NI * JB], f32)
            nc.sync.dma_start(cmat_sb[:], cmat[:])
            c_sb = cmat_sb[:, : JB * BH]
            rcol_sb = cmat_sb[:, JB * BH :]
            wsrc_sb = const.tile([NI, BH + NI * JB * P], f32)
            nc.sync.dma_start(wsrc_sb[:], wsrc[:])
            cpf_sb = wsrc_sb[:, :BH]
            rsel_sb = wsrc_sb[:, BH:]

            D = IG * JB * BH  # first pad column
            for ig in range(NI // IG):
                # staging for IG span rows: columns [il'][jb][b][h] + pad
                # (each STT writes one contiguous 1024-column block so the
                # subtile dep tracker sees disjoint regions)
                stag = stag_pool.tile([P, CW], f32)
                # Guard copy: writes only a pad column (read by the store
                # DMA, written by no STT). It advances the DVE vector clock
                # past the slot-release DMA tick and the const-DMA tick
                # without creating a same-engine WAW wait, so each STT below
                # carries exactly one sync wait (PE) — the walrus limit for
                # TensorScalarPtr.
                nc.vector.tensor_copy(stag[:, D : D + PADW], c_sb[:, 0:PADW])
                for il_ in range(IG):
                    il = ig * IG + il_
                    for jb in range(JB):
                        psum = psum_pool.tile([P, BH], f32)
                        g = il * JB + jb
                        lhsT = rsel_sb[:, g * P : (g + 1) * P]
                        for half in range(2):  # N<=512 per PSUM bank
                            nc.tensor.matmul(
                                psum[:, half * 512 : (half + 1) * 512],
                                lhsT,
                                cpf_sb[:, half * 512 : (half + 1) * 512],
                                start=True,
                                stop=True,
                            )
                        o0 = (il_ * JB + jb) * BH
                        nc.vector.scalar_tensor_tensor(
                            stag[:, o0 : o0 + BH],
                            c_sb[:, jb * BH : (jb + 1) * BH],
                            rcol_sb[:, g : g + 1],
                            psum[:],
                            Alu.mult,
                            Alu.subtract,
                        )
                nc.sync.dma_start(out[ig], stag[:])

    _strip_redundant_self_waits(nc)
    return nc


def _strip_redundant_self_waits(nc):
    """Drop causally-redundant self-engine waits on 2-wait instructions
    (walrus allows a single sync wait on the LW / STT / COPY structs).

    Two patterns, both from PSUM/staging slot reuse:
      - Matmult [PE>=a, DVE>=s]: the STT that freed the PSUM slot (tick s)
        itself waited on PE>=a, so DVE>=s implies PE>=a.
      - DVE op [DVE>=x, DMAHW*>=y]: the store DMA that freed the staging
        slot (tick y) waited on the DVE ops covered by x.
      - DMACopy [DVE>=k, DMAHW*>=y]: the DMAHW wait only orders the lane
        semaphore's previous +16; increments commute, HWDGE DMAs from one
        queue run FIFO, and the data dependency is the DVE wait.
    """
    _redistribute_tail_drain_waits(nc)
    for blk in nc.m.functions[0].blocks:
        for inst in blk.instructions:
            nm = type(inst).__name__
            si = inst.sync_info
            if si is None or len(si.on_wait) <= 1:
                continue
            if nm == "InstDMACopy":
                dve = [w for w in si.on_wait if w.ant_name.startswith("DVE")]
                rest = [w for w in si.on_wait if not w.ant_name.startswith("DVE")]
                assert len(dve) == 1 and len(rest) == 1, (nm, si)
                assert rest[0].ant_name.startswith("DMAHW"), si
                inst.sync_info = type(si)(on_wait=dve, on_update=si.on_update)
                continue
            if nm not in ("InstMatmult", "InstTensorCopy", "InstTensorScalarPtr"):
                continue
            eng = str(inst.engine).split(".")[-1]  # "PE" / "DVE"
            pre = {"PE": "PE", "DVE": "DVE", "ACT": "ACT"}[eng]
            selfw = [w for w in si.on_wait if w.ant_name.startswith(pre)]
            rest = [w for w in si.on_wait if not w.ant_name.startswith(pre)]
            assert len(selfw) == 1 and len(rest) == 1, (nm, si)
            if nm == "InstMatmult":
                assert rest[0].ant_name.startswith("DVE"), si
            else:
                assert rest[0].ant_name.startswith("DMAHW"), si
            inst.sync_info = type(si)(on_wait=rest, on_update=si.on_update)


def _redistribute_tail_drain_waits(nc):
    """Spread the kernel-tail drain's DMAHW lane waits one-per-drain.

    Tile's tail drain waits on PE, DVE and all 8 DMAHW lane sems (10 waits;
    walrus allows one sync wait per instruction). The PE/DVE waits are
    causally dominated by the lane waits (every store DMA's sequencer
    waited on the DVE tick, whose last op waited on PE). The 8 lane waits
    are distributed one each onto the tail drain and the per-engine drains
    of the two EVSEM barrier rounds that follow — all of which complete
    before the kernel's final gather — preserving "kernel completes only
    after all stores landed". The drains' existing `>=0` waits are no-op
    placeholders and may be replaced.
    """
    insts = []
    for blk in nc.m.functions[0].blocks:
        insts.extend(blk.instructions)
    tail = None
    for idx, inst in enumerate(insts):
        si = inst.sync_info
        if (
            type(inst).__name__ == "InstDrain"
            and si is not None
            and sum(w.ant_name.startswith("DMAHW") for w in si.on_wait) >= 2
        ):
            tail = idx
            break
    if tail is None:
        return
    d0 = insts[tail]
    si0 = d0.sync_info
    lanes = [w for w in si0.on_wait if w.ant_name.startswith("DMAHW")]
    carriers = [d0]
    for inst in insts[tail + 1 :]:
        if len(carriers) >= len(lanes):
            break
        if type(inst).__name__ != "InstDrain":
            continue
        si = inst.sync_info
        ok = si is None or all(w.wait_value == 0 for w in si.on_wait)
        if ok:
            carriers.append(inst)
    assert len(carriers) >= len(lanes), (len(carriers), len(lanes))
    for inst, lane_wait in zip(carriers, lanes):
        si = inst.sync_info
        upd = list(si.on_update) if si else []
        inst.sync_info = type(si0)(on_wait=[lane_wait], on_update=upd)


def _get_nc():
    if "nc" not in _cache:
        _cache["nc"] = _build_program()
    return _cache["nc"]


def _cumsum_f32(seq):
    """Mirror the reference's fp32 cumsum as closely as possible."""
    try:
        import jax
        import jax.numpy as jnp

        cpu = jax.devices("cpu")[0]
        with jax.default_device(cpu):
            return np.asarray(jnp.cumsum(jnp.asarray(seq), axis=1))
    except Exception:
        return np.cumsum(seq, axis=1, dtype=np.float32)


def _make_in_maps(seq):
    c = _cumsum_f32(seq)
    cp = np.zeros_like(c)
    cp[:, 1:] = c[:, :-1]

    # cmat[p, jb*BH + b*H + h] = c[b, jb*P+p, h] — identical on every core
    cmat = np.ascontiguousarray(
        c.transpose(1, 0, 2)
        .reshape(JB, P, BH)
        .transpose(1, 0, 2)
        .reshape(P, JB * BH)
    ).astype(np.float32)

    jv = np.arange(L)
    in_maps = []
    for k in range(NCORES):
        i0 = k * NI
        iv = i0 + np.arange(NI)
        cpf_k = np.ascontiguousarray(
            cp[:, i0 : i0 + NI].transpose(1, 0, 2).reshape(NI, BH)
        ).astype(np.float32)
        r_k = (
            1.0 / (np.abs(iv[:, None] - jv[None, :]).astype(np.float64) + 1.0)
        ).astype(np.float32)  # [NI, L]
        # rsel[k, (il*JB+jb)*P + m] = (k == il) * r_k[il, jb*P+m]
        rsel_k = np.zeros((NI, NI * JB * P), dtype=np.float32)
        for il in range(NI):
            rsel_k[il, il * L : (il + 1) * L] = r_k[il]
        wsrc_k = np.ascontiguousarray(np.concatenate([cpf_k, rsel_k], axis=1))
        # rcol[p, il*JB+jb] = r_k[il, jb*P+p]
        rcol_k = r_k.reshape(NI, JB, P).transpose(2, 0, 1).reshape(P, NI * JB)
        cmat_k = np.ascontiguousarray(np.concatenate([cmat, rcol_k], axis=1))
        in_maps.append({"cmat": cmat_k, "wsrc": wsrc_k})
    return in_maps


def _run(seq, trace=False, trace_kwargs=None):
    from concourse.bass_utils import run_bass_kernel_spmd

    nc = _get_nc()
    in_maps = _make_in_maps(seq)
    res = run_bass_kernel_spmd(
        nc,
        in_maps,
        core_ids=list(range(NCORES)),
        trace=trace,
        **(trace_kwargs or {}),
    )
    # per-core out: [NI//IG, P, CW]; cols il'*2048 + jb*1024 + b*256 + h
    allout = np.stack([r["out"] for r in res.results])  # [8, NI//IG, P, CW]
    allout = allout[:, :, :, : IG * JB * BH].reshape(
        NCORES, NI // IG, P, IG, JB, B, H
    )
    out = np.ascontiguousarray(
        allout.transpose(5, 0, 1, 3, 4, 2, 6)
    ).reshape(B, L, L, H)
    return out, res


def kernel(**inputs) -> np.ndarray:
    seq = np.asarray(inputs["seq_hiddens"], dtype=np.float32)
    assert seq.shape == (B, L, H), seq.shape
    out, _ = _run(seq)
    return out
